# revision 16
# baseline (speedup 1.0000x reference)
"""Trainium2 Bass kernel for BasicLSTM (B=64, T=512, D=U=512).

Sharding: data-parallel over batch across 8 cores (8 rows/core), weights
replicated; the sequential time scan runs locally per core.

Per-core strategy (everything unit-major / "transposed", all-SBUF):

  Phase A (input projection, zx.T = Wk.T @ x.T + b) is emitted chunk-by-
  chunk (64 timesteps) and OVERLAPPED with the scan: chunk c+2 is emitted
  after scan chunk c, so its DMA (GpSimd queue), xbar transposes (Sync
  queue), GEMM matmuls (PE idle slots) and DVE copy-outs fill engine idle
  time underneath the latency-bound scan without touching ACT.  The bias
  is applied by the DVE copy-out (tensor_scalar_add, per-partition
  scalar).  zx.T stays resident in SBUF as bf16 (128 KB/partition).

  Phase B: 512-step scan with zero DMA, structured to minimize the serial
  chain per step:
    - zx[t] enters PSUM via identity matmuls (start=True), no DVE add on
      the chain.
    - PSUM dependency tracking is BANK-granular, so each gate region
      (g / if / o, per unit-half) accumulates in its OWN psum bank; a
      region's ACT reader then waits only for that region's stop-MM.
    - MM order: kk{0,1} (needing only h half0) front-loaded; region stops
      complete in a ladder [g0, if0, g1, if1, o0, o1] matching the forced
      ACT order.  Both orders are pinned with same-engine dep edges,
      otherwise the scheduler bakes sim-time drift into the static
      schedule.
    - tails: tanh(g)/sig(if)/sig(o) ACT -> i*g, f*c, c=t1+t2 DVE ->
      tanh(c) ACT -> h=o*tc DVE.  h/c split per unit-half so half0's
      chain closes one matmul-block earlier than half1's.
"""

import numpy as np

B, T, D, U = 64, 512, 512, 512
G = 4 * U            # gates
P = 128              # partitions
N_CORES = 8
B_LOC = B // N_CORES  # 8
KD = D // P          # 4 k-tiles for x@Wk
KU = U // P          # 4 k-tiles for h@Wr
M = G // P           # 16 m-tiles of gates
TC = 64              # timesteps per phase-A chunk
NCH = T // TC        # 8 chunks
FB = M * B_LOC       # 128 free cols of z per step
HB = FB // 2         # 64 cols per half

# gate reordering: new m-tile order [i, f, o, g] -> original m-tile index
PERMM = list(range(8)) + [12, 13, 14, 15] + [8, 9, 10, 11]
# m-tiles per (half, class): m = 4a + q + 2h, a in [i,f,o,g], q in {0,1}
CLS_MS = [{a: [4 * a + 2 * h, 4 * a + 2 * h + 1] for a in range(4)}
          for h in range(2)]
# MM issue order (half, classes, kpair); see module docstring
MM_ORDER = [
    (0, [3], 0),        # g0 kk01    pairs 1-4
    (1, [3], 0),        # g1 kk01    5-8
    (0, [0, 1], 0),     # if0 kk01   9-16
    (0, [3], 1),        # g0 kk23    17-20 -> tanh(g0)
    (0, [0, 1], 1),     # if0 kk23   21-28 -> sig(if0)
    (1, [3], 1),        # g1 kk23    29-32 -> tanh(g1)
    (1, [0, 1], 0),     # if1 kk01   33-40
    (1, [0, 1], 1),     # if1 kk23   41-48 -> sig(if1)
    (0, [2], 0),        # o0 kk01    49-52
    (0, [2], 1),        # o0 kk23    53-56 -> sig(o0)
    (1, [2], 0),        # o1 kk01    57-60
    (1, [2], 1),        # o1 kk23    61-64 -> sig(o1)
]

_CACHE = {}


def _build(time_steps=T):
    import concourse.bacc as bacc
    import concourse.tile as tile
    import concourse.mybir as mybir
    from concourse import masks
    from bass_rust import add_dep_helper

    f32 = mybir.dt.float32
    bf16 = mybir.dt.bfloat16
    AF = mybir.ActivationFunctionType

    nc = bacc.Bacc(
        "TRN2",
        target_bir_lowering=False,
        debug=False,
        enable_asserts=True,
        num_devices=N_CORES,
    )

    x_h = nc.dram_tensor("x", [B_LOC, T, D], f32, kind="ExternalInput")
    wk_h = nc.dram_tensor("Wk", [D, G], f32, kind="ExternalInput")
    wr_h = nc.dram_tensor("Wr", [U, G], f32, kind="ExternalInput")
    b_h = nc.dram_tensor("b", [G], f32, kind="ExternalInput")
    out_h = nc.dram_tensor("h_last", [B_LOC, U], f32, kind="ExternalOutput")

    x_ap = x_h.ap()

    def load_weight_bf16(dst, src_h, stage_pool):
        """[512, 2048] fp32 weight -> dst bf16 [128, 64*128] laid out as
        (k, new_m) tiles of [128, 128] with the [i,f,o,g] gate reorder."""
        for k in range(KD):
            st = stage_pool.tile([P, G], f32, name="wstage", tag="wstage")
            nc.gpsimd.dma_start(st[:], src_h.ap()[k * P:(k + 1) * P, :])
            for nm0, om0, w in ((0, 0, 8), (8, 12, 4), (12, 8, 4)):
                nc.vector.tensor_copy(
                    dst[:, (k * M + nm0) * P:(k * M + nm0 + w) * P],
                    st[:, om0 * P:(om0 + w) * P],
                )

    with tile.TileContext(nc) as tc:
        with (
            tc.tile_pool(name="persist", bufs=1) as persist_pool,
            tc.tile_pool(name="wk", bufs=1) as wk_pool,
            tc.tile_pool(name="wr", bufs=1) as wr_pool,
            tc.tile_pool(name="state", bufs=1) as st_pool,
        ):
            # zx.T resident in SBUF: col = m*(T*8) + b*64 + t  (bf16)
            zxT = persist_pool.tile([P, T * FB], bf16)
            zxT4 = zxT.rearrange("p (m b t) -> p m b t", m=M, b=B_LOC)
            b_sb = persist_pool.tile([P, M], f32)
            nc.sync.dma_start(b_sb[:], b_h.ap().rearrange("(m p) -> p m", p=P))
            ident = persist_pool.tile([P, P], bf16)
            masks.make_identity(nc, ident[:])

            wk_sb = wk_pool.tile([P, KD * G], bf16)
            wr_sb = wr_pool.tile([P, KU * G], bf16)
            with tc.tile_pool(name="stage", bufs=2) as stage_pool:
                load_weight_bf16(wk_sb, wk_h, stage_pool)
                load_weight_bf16(wr_sb, wr_h, stage_pool)

            # scan state: h bf16, c fp32, per (parity, half)
            hs = [[st_pool.tile([P, 2 * B_LOC], bf16, name=f"h{i}{j}")
                   for j in range(2)] for i in range(2)]
            cs = [[st_pool.tile([P, 2 * B_LOC], f32, name=f"c{i}{j}")
                   for j in range(2)] for i in range(2)]
            for j in range(2):
                nc.vector.memset(hs[0][j][:], 0.0)
                nc.vector.memset(cs[0][j][:], 0.0)
            hf = st_pool.tile([P, KU * B_LOC], f32, name="hf")

            with (
                tc.tile_pool(name="nat", bufs=1) as nat_pool,
                tc.tile_pool(name="xtb", bufs=1) as xtb_pool,
                tc.tile_pool(name="gemm_psum", bufs=2, space="PSUM") as gps_pool,
                tc.tile_pool(name="gates", bufs=2) as gate_pool,
                tc.tile_pool(name="tmp", bufs=2) as tmp_pool,
                tc.tile_pool(name="scan_psum", bufs=1, space="PSUM") as sps_pool,
            ):

                def phase_a_chunk(chunk):
                    t0 = chunk * TC
                    natbs = []
                    for bp in range(4):
                        nat = nat_pool.tile([P, D], f32, name="nat",
                                            tag=f"nat{bp}")
                        for j in range(2):
                            nc.gpsimd.dma_start(
                                nat[j * TC:(j + 1) * TC, :],
                                x_ap[2 * bp + j, t0:t0 + TC, :],
                            )
                        natb = nat_pool.tile([P, D], bf16, name="natb",
                                             tag=f"natb{bp}")
                        nc.vector.tensor_copy(natb[:], nat[:])
                        natbs.append(natb)
                    xtbs = []
                    for k in range(KD):
                        xtb = xtb_pool.tile([P, TC * B_LOC], bf16,
                                            name=f"xtb{k}", tag=f"xtb{k}")
                        for bp in range(4):
                            nc.sync.dma_start(
                                xtb[:, bp * P:(bp + 1) * P],
                                natbs[bp][:, k * P:(k + 1) * P],
                                transpose=True,
                            )
                        xtbs.append(xtb)
                    for m in range(M):
                        ps = gps_pool.tile([P, TC * B_LOC], f32,
                                           name="gps", tag="gps")
                        for k in range(KD):
                            nc.tensor.matmul(
                                ps[:],
                                wk_sb[:, (k * M + m) * P:(k * M + m + 1) * P],
                                xtbs[k][:],
                                start=(k == 0),
                                stop=(k == KD - 1),
                            )
                        # copy-out + per-partition bias on DVE (keeps ACT
                        # free for the scan)
                        nc.vector.tensor_scalar_add(
                            zxT4[:, m, :, t0:t0 + TC],
                            ps.rearrange("p (b t) -> p b t", t=TC)[:],
                            b_sb[:, PERMM[m]:PERMM[m] + 1],
                        )

                def scan_steps(ts):
                    for t in ts:
                        pp = t % 2
                        qq = 1 - pp
                        h_prev = hs[pp]
                        last = t == time_steps - 1

                        ps_g = [sps_pool.tile([P, 16], f32, name=f"psg{hf_}",
                                              tag=f"psg{hf_}")
                                for hf_ in range(2)]
                        ps_if = [sps_pool.tile([P, 32], f32, name=f"psif{hf_}",
                                               tag=f"psif{hf_}")
                                 for hf_ in range(2)]
                        ps_o = [sps_pool.tile([P, 16], f32, name=f"pso{hf_}",
                                              tag=f"pso{hf_}")
                                for hf_ in range(2)]
                        reg_tile = {3: ps_g, 0: ps_if, 1: ps_if, 2: ps_o}
                        reg_off = {3: 0, 0: 0, 1: 16, 2: 0}
                        zxh = [
                            (zxT4
                             .rearrange("p (a qq) b t -> p a qq b t", qq=4)
                             [:, :, 2 * half:2 * half + 2, :, t])
                            for half in range(2)
                        ]
                        for half in range(2):
                            for tile_, a_lo, a_hi in ((ps_g[half], 3, 4),
                                                      (ps_if[half], 0, 2),
                                                      (ps_o[half], 2, 3)):
                                nc.tensor.matmul(
                                    tile_.rearrange("p (a q b) -> p a q b",
                                                    q=2, b=B_LOC)[:],
                                    ident[:],
                                    zxh[half][:, a_lo:a_hi],
                                    start=True,
                                    stop=False,
                                    skip_group_check=True,
                                )
                        n_left = [[8, 8, 8, 8] for _ in range(2)]
                        prev_mm = None
                        for half, classes, kpair in MM_ORDER:
                            for a in classes:
                                for m in CLS_MS[half][a]:
                                    q = m % 4 - 2 * half
                                    dst = reg_tile[a][half][
                                        :, reg_off[a] + q * 8:
                                        reg_off[a] + q * 8 + 8]
                                    for kk in (2 * kpair, 2 * kpair + 1):
                                        n_left[half][a] -= 1
                                        grp_left = (
                                            n_left[half][0] + n_left[half][1]
                                            if a in (0, 1)
                                            else n_left[half][a])
                                        i_mm = nc.tensor.matmul(
                                            dst,
                                            wr_sb[:, (kk * M + m) * P:
                                                  (kk * M + m + 1) * P],
                                            h_prev[kk // 2][
                                                :, (kk % 2) * B_LOC:
                                                (kk % 2 + 1) * B_LOC],
                                            start=False,
                                            stop=(grp_left == 0),
                                            skip_group_check=True,
                                        )
                                        if prev_mm is not None:
                                            add_dep_helper(
                                                i_mm.ins, prev_mm.ins,
                                                reason="mm order")
                                        prev_mm = i_mm

                        # gates: ACT ladder order forced to match stops
                        gts = [gate_pool.tile([P, HB], f32, name=f"gt{half}",
                                              tag=f"gt{half}")
                               for half in range(2)]
                        ladder = [
                            nc.scalar.activation(gts[0][:, 48:64], ps_g[0][:],
                                                 AF.Tanh),       # tanh g0
                            nc.scalar.activation(gts[0][:, 0:32], ps_if[0][:],
                                                 AF.Sigmoid),    # sig if0
                            nc.scalar.activation(gts[1][:, 48:64], ps_g[1][:],
                                                 AF.Tanh),       # tanh g1
                            nc.scalar.activation(gts[1][:, 0:32], ps_if[1][:],
                                                 AF.Sigmoid),    # sig if1
                            nc.scalar.activation(gts[0][:, 32:48], ps_o[0][:],
                                                 AF.Sigmoid),    # sig o0
                            nc.scalar.activation(gts[1][:, 32:48], ps_o[1][:],
                                                 AF.Sigmoid),    # sig o1
                        ]
                        for a_, b_ in zip(ladder[1:], ladder):
                            add_dep_helper(a_.ins, b_.ins, reason="act ladder")

                        tcs = []
                        for half in range(2):
                            gt = gts[half]
                            t2 = tmp_pool.tile([P, 2 * B_LOC], f32,
                                               name=f"t2{half}",
                                               tag=f"t2{half}")
                            nc.vector.tensor_mul(t2[:], gt[:, 0:16],
                                                 gt[:, 48:64])
                            t1 = tmp_pool.tile([P, 2 * B_LOC], f32,
                                               name=f"t1{half}",
                                               tag=f"t1{half}")
                            nc.vector.tensor_mul(t1[:], gt[:, 16:32],
                                                 cs[pp][half][:])
                            nc.vector.tensor_add(cs[qq][half][:], t1[:], t2[:])
                            tc_t = tmp_pool.tile([P, 2 * B_LOC], f32,
                                                 name=f"tc{half}",
                                                 tag=f"tc{half}")
                            nc.scalar.activation(tc_t[:], cs[qq][half][:],
                                                 AF.Tanh)
                            tcs.append(tc_t)
                        for half in range(2):
                            if last:
                                nc.vector.tensor_mul(
                                    hf[:, half * 16:(half + 1) * 16],
                                    gts[half][:, 32:48], tcs[half][:],
                                )
                            else:
                                nc.vector.tensor_mul(hs[qq][half][:],
                                                     gts[half][:, 32:48],
                                                     tcs[half][:])

                # interleaved emission: A(c+2) after scan chunk c, so its
                # ops yield (priority) to the running scan chunk but are
                # done before scan chunk c+2 consumes them
                phase_a_chunk(0)
                phase_a_chunk(1)
                for c in range(NCH):
                    scan_steps(range(c * TC, min((c + 1) * TC, time_steps)))
                    if c + 2 < NCH:
                        phase_a_chunk(c + 2)

                for kk in range(KU):
                    nc.sync.dma_start(
                        out_h.ap()[:, kk * P:(kk + 1) * P]
                        .rearrange("b p -> p b"),
                        hf[:, kk * B_LOC:(kk + 1) * B_LOC],
                    )

    nc.compile()
    return nc


def _get_nc(time_steps=T):
    key = time_steps
    if key not in _CACHE:
        _CACHE[key] = _build(time_steps)
    return _CACHE[key]


def kernel(x, Wk, Wr, b):
    from concourse import bass_utils

    x = np.ascontiguousarray(np.asarray(x, dtype=np.float32))
    Wk = np.ascontiguousarray(np.asarray(Wk, dtype=np.float32))
    Wr = np.ascontiguousarray(np.asarray(Wr, dtype=np.float32))
    b = np.ascontiguousarray(np.asarray(b, dtype=np.float32))

    nc = _get_nc(T)
    in_maps = [
        {
            "x": x[c * B_LOC:(c + 1) * B_LOC],
            "Wk": Wk,
            "Wr": Wr,
            "b": b,
        }
        for c in range(N_CORES)
    ]
    res = bass_utils.run_bass_kernel_spmd(nc, in_maps, core_ids=list(range(N_CORES)))
    return np.concatenate([res.results[c]["h_last"] for c in range(N_CORES)], axis=0)


# revision 17
# speedup vs baseline: 1.1689x; 1.1689x over previous
"""Trainium2 Bass kernel for BasicLSTM (B=64, T=512, D=U=512).

Sharding: data-parallel over batch across 8 cores (8 rows/core), weights
replicated; the sequential time scan runs locally per core.

Per-core strategy (everything unit-major / "transposed", all-SBUF):

  Phase A (input projection, zx.T = Wk.T @ x.T + b) is emitted chunk-by-
  chunk (64 timesteps) and OVERLAPPED with the scan: chunk c+2 is emitted
  after scan chunk c, so its DMA (GpSimd queue), xbar transposes (Sync
  queue), GEMM matmuls (PE idle slots) and DVE copy-outs fill engine idle
  time underneath the latency-bound scan without touching ACT.  The bias
  is applied by the DVE copy-out (tensor_scalar_add, per-partition
  scalar).  zx.T stays resident in SBUF as bf16 (128 KB/partition).

  Phase B: 512-step scan with zero DMA, structured to minimize the serial
  chain per step:
    - zx[t] enters PSUM via identity matmuls (start=True), no DVE add on
      the chain.
    - PSUM dependency tracking is BANK-granular, so each gate region
      (g / if / o, per unit-half) accumulates in its OWN psum bank; a
      region's ACT reader then waits only for that region's stop-MM.
    - MM order: kk{0,1} (needing only h half0) front-loaded; region stops
      complete in a ladder [g0, if0, g1, if1, o0, o1] matching the forced
      ACT order.  Both orders are pinned with same-engine dep edges,
      otherwise the scheduler bakes sim-time drift into the static
      schedule.
    - tails: tanh(g)/sig(if)/sig(o) ACT -> i*g, f*c, c=t1+t2 DVE ->
      tanh(c) ACT -> h=o*tc DVE.  h/c split per unit-half so half0's
      chain closes one matmul-block earlier than half1's.
"""

import numpy as np

B, T, D, U = 64, 512, 512, 512
G = 4 * U            # gates
P = 128              # partitions
N_CORES = 8
B_LOC = B // N_CORES  # 8
KD = D // P          # 4 k-tiles for x@Wk
KU = U // P          # 4 k-tiles for h@Wr
M = G // P           # 16 m-tiles of gates
TC = 64              # timesteps per phase-A chunk
NCH = T // TC        # 8 chunks
FB = M * B_LOC       # 128 free cols of z per step
HB = FB // 2         # 64 cols per half

# gate reordering: new m-tile order [i, f, o, g] -> original m-tile index
PERMM = list(range(8)) + [12, 13, 14, 15] + [8, 9, 10, 11]
# m-tiles per (half, class): m = 4a + q + 2h, a in [i,f,o,g], q in {0,1}
CLS_MS = [{a: [4 * a + 2 * h, 4 * a + 2 * h + 1] for a in range(4)}
          for h in range(2)]
# MM issue order (half, classes, kpair); see module docstring
MM_ORDER = [
    (0, [3], 0),        # g0 kk01    pairs 1-4
    (1, [3], 0),        # g1 kk01    5-8
    (0, [0, 1], 0),     # if0 kk01   9-16
    (0, [3], 1),        # g0 kk23    17-20 -> tanh(g0)
    (0, [0, 1], 1),     # if0 kk23   21-28 -> sig(if0)
    (1, [3], 1),        # g1 kk23    29-32 -> tanh(g1)
    (1, [0, 1], 0),     # if1 kk01   33-40
    (1, [0, 1], 1),     # if1 kk23   41-48 -> sig(if1)
    (0, [2], 0),        # o0 kk01    49-52
    (0, [2], 1),        # o0 kk23    53-56 -> sig(o0)
    (1, [2], 0),        # o1 kk01    57-60
    (1, [2], 1),        # o1 kk23    61-64 -> sig(o1)
]

_CACHE = {}


def _build(time_steps=T):
    import concourse.bacc as bacc
    import concourse.tile as tile
    import concourse.mybir as mybir
    from concourse import masks
    from bass_rust import add_dep_helper

    f32 = mybir.dt.float32
    bf16 = mybir.dt.bfloat16
    AF = mybir.ActivationFunctionType

    nc = bacc.Bacc(
        "TRN2",
        target_bir_lowering=False,
        debug=False,
        enable_asserts=True,
        num_devices=N_CORES,
    )

    x_h = nc.dram_tensor("x", [B_LOC, T, D], f32, kind="ExternalInput")
    wk_h = nc.dram_tensor("Wk", [D, G], f32, kind="ExternalInput")
    wr_h = nc.dram_tensor("Wr", [U, G], f32, kind="ExternalInput")
    b_h = nc.dram_tensor("b", [G], f32, kind="ExternalInput")
    out_h = nc.dram_tensor("h_last", [B_LOC, U], f32, kind="ExternalOutput")

    x_ap = x_h.ap()

    def load_weight_bf16(dst, src_h, stage_pool):
        """[512, 2048] fp32 weight -> dst bf16 [128, 64*128] laid out as
        (k, new_m) tiles of [128, 128] with the [i,f,o,g] gate reorder."""
        for k in range(KD):
            st = stage_pool.tile([P, G], f32, name="wstage", tag="wstage")
            nc.gpsimd.dma_start(st[:], src_h.ap()[k * P:(k + 1) * P, :])
            for nm0, om0, w in ((0, 0, 8), (8, 12, 4), (12, 8, 4)):
                nc.vector.tensor_copy(
                    dst[:, (k * M + nm0) * P:(k * M + nm0 + w) * P],
                    st[:, om0 * P:(om0 + w) * P],
                )

    with tile.TileContext(nc) as tc:
        with (
            tc.tile_pool(name="persist", bufs=1) as persist_pool,
            tc.tile_pool(name="wk", bufs=1) as wk_pool,
            tc.tile_pool(name="wr", bufs=1) as wr_pool,
            tc.tile_pool(name="state", bufs=1) as st_pool,
        ):
            # zx.T resident in SBUF: col = m*(T*8) + b*64 + t  (bf16)
            zxT = persist_pool.tile([P, T * FB], bf16)
            zxT4 = zxT.rearrange("p (m b t) -> p m b t", m=M, b=B_LOC)
            b_sb = persist_pool.tile([P, M], f32)
            nc.sync.dma_start(b_sb[:], b_h.ap().rearrange("(m p) -> p m", p=P))
            ident = persist_pool.tile([P, P], bf16)
            masks.make_identity(nc, ident[:])

            wk_sb = wk_pool.tile([P, KD * G], bf16)
            wr_sb = wr_pool.tile([P, KU * G], bf16)
            with tc.tile_pool(name="stage", bufs=2) as stage_pool:
                load_weight_bf16(wk_sb, wk_h, stage_pool)
                load_weight_bf16(wr_sb, wr_h, stage_pool)

            # scan state: h bf16, c fp32, per (parity, half)
            hs = [[st_pool.tile([P, 2 * B_LOC], bf16, name=f"h{i}{j}")
                   for j in range(2)] for i in range(2)]
            cs = [[st_pool.tile([P, 2 * B_LOC], f32, name=f"c{i}{j}")
                   for j in range(2)] for i in range(2)]
            for j in range(2):
                nc.vector.memset(hs[0][j][:], 0.0)
                nc.vector.memset(cs[0][j][:], 0.0)
            hf = st_pool.tile([P, KU * B_LOC], f32, name="hf")

            with (
                tc.tile_pool(name="nat", bufs=1) as nat_pool,
                tc.tile_pool(name="xtb", bufs=1) as xtb_pool,
                tc.tile_pool(name="gemm_psum", bufs=2, space="PSUM") as gps_pool,
                tc.tile_pool(name="gates", bufs=2) as gate_pool,
                tc.tile_pool(name="tmp", bufs=2) as tmp_pool,
                tc.tile_pool(name="scan_psum", bufs=1, space="PSUM") as sps_pool,
            ):

                def phase_a_chunk(chunk):
                    # low priority: phase-A ops only fill engine idle time
                    # underneath the latency-bound scan
                    prio_save = tc.cur_priority
                    tc.cur_priority = prio_save + 500_000
                    t0 = chunk * TC
                    natbs = []
                    for bp in range(4):
                        nat = nat_pool.tile([P, D], f32, name="nat",
                                            tag=f"nat{bp}")
                        for j in range(2):
                            nc.gpsimd.dma_start(
                                nat[j * TC:(j + 1) * TC, :],
                                x_ap[2 * bp + j, t0:t0 + TC, :],
                            )
                        natb = nat_pool.tile([P, D], bf16, name="natb",
                                             tag=f"natb{bp}")
                        nc.vector.tensor_copy(natb[:], nat[:])
                        natbs.append(natb)
                    xtbs = []
                    for k in range(KD):
                        xtb = xtb_pool.tile([P, TC * B_LOC], bf16,
                                            name=f"xtb{k}", tag=f"xtb{k}")
                        for bp in range(4):
                            nc.sync.dma_start(
                                xtb[:, bp * P:(bp + 1) * P],
                                natbs[bp][:, k * P:(k + 1) * P],
                                transpose=True,
                            )
                        xtbs.append(xtb)
                    for m in range(M):
                        ps = gps_pool.tile([P, TC * B_LOC], f32,
                                           name="gps", tag="gps")
                        for k in range(KD):
                            nc.tensor.matmul(
                                ps[:],
                                wk_sb[:, (k * M + m) * P:(k * M + m + 1) * P],
                                xtbs[k][:],
                                start=(k == 0),
                                stop=(k == KD - 1),
                            )
                        # copy-out + per-partition bias on DVE (keeps ACT
                        # free); split so each op blocks the DVE only
                        # briefly when the scan chain needs it
                        ps3 = ps.rearrange("p (b t) -> p b t", t=TC)
                        for bh in range(2):
                            nc.vector.tensor_scalar_add(
                                zxT4[:, m, bh * 4:(bh + 1) * 4, t0:t0 + TC],
                                ps3[:, bh * 4:(bh + 1) * 4],
                                b_sb[:, PERMM[m]:PERMM[m] + 1],
                            )
                    tc.cur_priority = prio_save

                def scan_steps(ts):
                    for t in ts:
                        pp = t % 2
                        qq = 1 - pp
                        h_prev = hs[pp]
                        last = t == time_steps - 1

                        ps_g = [sps_pool.tile([P, 16], f32, name=f"psg{hf_}",
                                              tag=f"psg{hf_}")
                                for hf_ in range(2)]
                        ps_if = [sps_pool.tile([P, 32], f32, name=f"psif{hf_}",
                                               tag=f"psif{hf_}")
                                 for hf_ in range(2)]
                        ps_o = [sps_pool.tile([P, 16], f32, name=f"pso{hf_}",
                                              tag=f"pso{hf_}")
                                for hf_ in range(2)]
                        reg_tile = {3: ps_g, 0: ps_if, 1: ps_if, 2: ps_o}
                        reg_off = {3: 0, 0: 0, 1: 16, 2: 0}
                        zxh = [
                            (zxT4
                             .rearrange("p (a qq) b t -> p a qq b t", qq=4)
                             [:, :, 2 * half:2 * half + 2, :, t])
                            for half in range(2)
                        ]
                        for half in range(2):
                            for tile_, a_lo, a_hi in ((ps_g[half], 3, 4),
                                                      (ps_if[half], 0, 2),
                                                      (ps_o[half], 2, 3)):
                                nc.tensor.matmul(
                                    tile_.rearrange("p (a q b) -> p a q b",
                                                    q=2, b=B_LOC)[:],
                                    ident[:],
                                    zxh[half][:, a_lo:a_hi],
                                    start=True,
                                    stop=False,
                                    skip_group_check=True,
                                )
                        n_left = [[8, 8, 8, 8] for _ in range(2)]
                        prev_mm = None
                        for half, classes, kpair in MM_ORDER:
                            for a in classes:
                                for m in CLS_MS[half][a]:
                                    q = m % 4 - 2 * half
                                    dst = reg_tile[a][half][
                                        :, reg_off[a] + q * 8:
                                        reg_off[a] + q * 8 + 8]
                                    for kk in (2 * kpair, 2 * kpair + 1):
                                        n_left[half][a] -= 1
                                        grp_left = (
                                            n_left[half][0] + n_left[half][1]
                                            if a in (0, 1)
                                            else n_left[half][a])
                                        i_mm = nc.tensor.matmul(
                                            dst,
                                            wr_sb[:, (kk * M + m) * P:
                                                  (kk * M + m + 1) * P],
                                            h_prev[kk // 2][
                                                :, (kk % 2) * B_LOC:
                                                (kk % 2 + 1) * B_LOC],
                                            start=False,
                                            stop=(grp_left == 0),
                                            skip_group_check=True,
                                        )
                                        if prev_mm is not None:
                                            add_dep_helper(
                                                i_mm.ins, prev_mm.ins,
                                                reason="mm order")
                                        prev_mm = i_mm

                        # gates: ACT ladder order forced to match stops
                        gts = [gate_pool.tile([P, HB], f32, name=f"gt{half}",
                                              tag=f"gt{half}")
                               for half in range(2)]
                        ladder = [
                            nc.scalar.activation(gts[0][:, 48:64], ps_g[0][:],
                                                 AF.Tanh),       # tanh g0
                            nc.scalar.activation(gts[0][:, 0:32], ps_if[0][:],
                                                 AF.Sigmoid),    # sig if0
                            nc.scalar.activation(gts[1][:, 48:64], ps_g[1][:],
                                                 AF.Tanh),       # tanh g1
                            nc.scalar.activation(gts[1][:, 0:32], ps_if[1][:],
                                                 AF.Sigmoid),    # sig if1
                            nc.scalar.activation(gts[0][:, 32:48], ps_o[0][:],
                                                 AF.Sigmoid),    # sig o0
                            nc.scalar.activation(gts[1][:, 32:48], ps_o[1][:],
                                                 AF.Sigmoid),    # sig o1
                        ]
                        for a_, b_ in zip(ladder[1:], ladder):
                            add_dep_helper(a_.ins, b_.ins, reason="act ladder")

                        tcs = []
                        for half in range(2):
                            gt = gts[half]
                            t2 = tmp_pool.tile([P, 2 * B_LOC], f32,
                                               name=f"t2{half}",
                                               tag=f"t2{half}")
                            nc.vector.tensor_mul(t2[:], gt[:, 0:16],
                                                 gt[:, 48:64])
                            t1 = tmp_pool.tile([P, 2 * B_LOC], f32,
                                               name=f"t1{half}",
                                               tag=f"t1{half}")
                            nc.vector.tensor_mul(t1[:], gt[:, 16:32],
                                                 cs[pp][half][:])
                            nc.vector.tensor_add(cs[qq][half][:], t1[:], t2[:])
                            tc_t = tmp_pool.tile([P, 2 * B_LOC], f32,
                                                 name=f"tc{half}",
                                                 tag=f"tc{half}")
                            nc.scalar.activation(tc_t[:], cs[qq][half][:],
                                                 AF.Tanh)
                            tcs.append(tc_t)
                        for half in range(2):
                            if last:
                                nc.vector.tensor_mul(
                                    hf[:, half * 16:(half + 1) * 16],
                                    gts[half][:, 32:48], tcs[half][:],
                                )
                            else:
                                nc.vector.tensor_mul(hs[qq][half][:],
                                                     gts[half][:, 32:48],
                                                     tcs[half][:])

                # interleaved emission: A(c+2) after scan chunk c, so its
                # ops yield (priority) to the running scan chunk but are
                # done before scan chunk c+2 consumes them
                phase_a_chunk(0)
                phase_a_chunk(1)
                for c in range(NCH):
                    scan_steps(range(c * TC, min((c + 1) * TC, time_steps)))
                    if c + 2 < NCH:
                        phase_a_chunk(c + 2)

                for kk in range(KU):
                    nc.sync.dma_start(
                        out_h.ap()[:, kk * P:(kk + 1) * P]
                        .rearrange("b p -> p b"),
                        hf[:, kk * B_LOC:(kk + 1) * B_LOC],
                    )

    nc.compile()
    return nc


def _get_nc(time_steps=T):
    key = time_steps
    if key not in _CACHE:
        _CACHE[key] = _build(time_steps)
    return _CACHE[key]


def kernel(x, Wk, Wr, b):
    from concourse import bass_utils

    x = np.ascontiguousarray(np.asarray(x, dtype=np.float32))
    Wk = np.ascontiguousarray(np.asarray(Wk, dtype=np.float32))
    Wr = np.ascontiguousarray(np.asarray(Wr, dtype=np.float32))
    b = np.ascontiguousarray(np.asarray(b, dtype=np.float32))

    nc = _get_nc(T)
    in_maps = [
        {
            "x": x[c * B_LOC:(c + 1) * B_LOC],
            "Wk": Wk,
            "Wr": Wr,
            "b": b,
        }
        for c in range(N_CORES)
    ]
    res = bass_utils.run_bass_kernel_spmd(nc, in_maps, core_ids=list(range(N_CORES)))
    return np.concatenate([res.results[c]["h_last"] for c in range(N_CORES)], axis=0)
